# revision 15
# baseline (speedup 1.0000x reference)
"""Trainium2 Bass kernel for nn_BatchedSpGat (2-layer GAT + L2-normalize + relu).

Strategy (8 NeuronCores, SPMD single program):
  - Nodes sharded contiguously: core c owns nodes [c*NPC, (c+1)*NPC).
  - Edges assigned to the owner of their DST node, sorted/grouped by
    (dst-tile-of-128, src-half), padded so every (dst-tile, half) group is a
    fixed number of 128-edge tiles (uniform across cores -> one SPMD program).
  - Per layer: sharded GEMM (own nodes) -> AllGather of a per-node table
    [h | al_src | al_dst | pad] -> per-edge gather of h rows (dma_gather,
    int16 idx, hence the src-half split at 32768) + tiny indirect gathers of
    al terms -> per-128-edge-tile one-hot matmul (lhsT = (dst==iota) mask)
    accumulating the segment-softmax numerator/denominator in PSUM ->
    per-dst-tile normalize.
  - Softmax uses no max-subtraction (logits empirically bounded ~14; exp is
    safe in fp32; alpha is shift-invariant so the result is identical).

kernel(**inputs) takes the FULL problem inputs and returns the FULL output.
"""
import os
import sys
import time
from contextlib import ExitStack

import numpy as np

for _p in ('/opt/trn_rl_repo', '/root/.axon_site/_ro/trn_rl_repo'):
    if os.path.isdir(_p) and _p not in sys.path:
        sys.path.insert(0, _p)

import concourse.bass as bass
import concourse.bacc as bacc
import concourse.tile as tile
import concourse.mybir as mybir
from concourse.bass import AP, IndirectOffsetOnAxis
from concourse.bass_utils import run_bass_kernel_spmd
from concourse.library_config import mlp as _mlp_lib
from concourse.masks import make_identity

F32 = mybir.dt.float32
I16 = mybir.dt.int16
I32 = mybir.dt.int32
OP = mybir.AluOpType
AF = mybir.ActivationFunctionType

NEG_SLOPE = 0.2


class Cfg:
    def __init__(self, N=50000, E=800000, cores=8, half=32768,
                 F0=512, F1=128, H1=4, F2=256, CD=2):
        self.N = N                  # nodes
        self.E = E                  # edges (before self-loops)
        self.CORES = cores
        self.HALF = half            # src-half split for int16 gather idx
        self.F0 = F0                # input features
        self.F1 = F1                # layer-1 out features (H1 * C1)
        self.H1 = H1                # layer-1 heads
        self.C1 = F1 // H1
        self.F2 = F2                # layer-2 out features (1 head)
        self.CD = CD                # dst-tiles per gather chunk
        assert N % cores == 0
        self.NPC = N // cores       # nodes per core
        self.DT = (self.NPC + 127) // 128   # dst tiles per core
        self.KT = F0 // 128         # k-tiles for GEMM1
        # table strides (multiples of 64 floats for the 256B dma_gather
        # stride constraint). table1: [h1(F1) | als1(H1) | pad]
        self.ST1 = ((F1 + H1 + 63) // 64) * 64
        # table2: [h2(F2) | als2(1) | pad]
        self.ST2 = ((F2 + 1 + 63) // 64) * 64
        self.ALS = 64               # al_own row width (ald | pad)


# ---------------------------------------------------------------------------
# Host-side preprocessing
# ---------------------------------------------------------------------------

def preprocess(edge_index, cfg: Cfg):
    """Partition/pad edges. Returns (percore, T_LO, T_HI).

    percore[c] holds, for each stream s in ('lo','hi'):
      gidx_{s}   int16  [128, NSLOT*8]  wrapped gather indices (16-row layout)
      didx_{s}   int16  [128, NSLOT*8]  wrapped dst_local indices (al_d gather)
      dstrel_{s} f32    [128, NSLOT]    dst_local - tile*128 (-1 for dummies)
    where NSLOT = DT * T_S (tile slot count), edge i of the stream lives at
    [i%128, i//128] (and [i%16, i//16] for the wrapped gidx).
    """
    N, NPC, DT, HALF, CORES = cfg.N, cfg.NPC, cfg.DT, cfg.HALF, cfg.CORES
    src = np.concatenate([np.asarray(edge_index[0], np.int64),
                          np.arange(N, dtype=np.int64)])
    dst = np.concatenate([np.asarray(edge_index[1], np.int64),
                          np.arange(N, dtype=np.int64)])
    owner = dst // NPC

    groups = []
    for c in range(CORES):
        m = owner == c
        s_c, d_c = src[m], dst[m]
        dl = d_c - c * NPC
        dt = dl // 128
        order = np.argsort(dt, kind='stable')
        s_c, d_c, dl_c, dt_c = s_c[order], d_c[order], dl[order], dt[order]
        lo = s_c < HALF
        bounds = np.searchsorted(dt_c, np.arange(DT + 1))
        groups.append((s_c, d_c, dl_c, lo, bounds))

    def tiles_needed(c, t, want_lo):
        s_c, d_c, dl_c, lo, bounds = groups[c]
        sl = slice(bounds[t], bounds[t + 1])
        k = int(np.count_nonzero(lo[sl] == want_lo))
        return (k + 127) // 128

    T_LO = max(1, max(tiles_needed(c, t, True)
                      for c in range(CORES) for t in range(DT)))
    T_HI = max(1, max(tiles_needed(c, t, False)
                      for c in range(CORES) for t in range(DT)))

    percore = []
    for c in range(CORES):
        s_c, d_c, dl_c, lo, bounds = groups[c]
        arrs = {}
        for tag, want_lo, T_S in (('lo', True, T_LO), ('hi', False, T_HI)):
            nslot = DT * T_S
            tot = nslot * 128
            gidx = np.zeros(tot, np.int16)
            didx = np.zeros(tot, np.int16)
            drel = np.full(tot, -1.0, np.float32)
            for t in range(DT):
                sl = slice(bounds[t], bounds[t + 1])
                m = lo[sl] == want_lo
                s_t = s_c[sl][m]
                d_t = d_c[sl][m]
                dl_t = dl_c[sl][m]
                k = len(s_t)
                o = t * T_S * 128
                gidx[o:o + k] = (s_t - (0 if want_lo else HALF)).astype(np.int16)
                didx[o:o + k] = dl_t.astype(np.int16)
                drel[o:o + k] = (dl_t - t * 128).astype(np.float32)
            w16 = gidx.reshape(-1, 16).T                      # [16, tot/16]
            arrs['gidx_' + tag] = np.ascontiguousarray(np.tile(w16, (8, 1)))
            d16 = didx.reshape(-1, 16).T
            arrs['didx_' + tag] = np.ascontiguousarray(np.tile(d16, (8, 1)))
            arrs['dstrel_' + tag] = np.ascontiguousarray(drel.reshape(nslot, 128).T)
        percore.append(arrs)
    return percore, T_LO, T_HI


def make_in_maps(inputs, cfg: Cfg, percore, T_LO, T_HI):
    N, NPC, F0, F1, H1, F2 = cfg.N, cfg.NPC, cfg.F0, cfg.F1, cfg.H1, cfg.F2
    x = np.asarray(inputs['x'], np.float32).reshape(N, F0)
    W1 = np.ascontiguousarray(np.asarray(inputs['W1'], np.float32))
    W2 = np.ascontiguousarray(np.asarray(inputs['W2'], np.float32))
    a1s = np.asarray(inputs['a1_s'], np.float32).reshape(1, F1)
    a1d = np.asarray(inputs['a1_d'], np.float32).reshape(1, F1)
    a2s = np.asarray(inputs['a2_s'], np.float32).reshape(1, F2)
    a2d = np.asarray(inputs['a2_d'], np.float32).reshape(1, F2)
    b1 = np.asarray(inputs['b1'], np.float32).reshape(1, F1)
    b2 = np.asarray(inputs['b2'], np.float32).reshape(1, F2)

    shared = {
        'W1': W1,
        'W2': W2,
        'a1s_rep': np.ascontiguousarray(np.tile(a1s, (128, 1))),
        'a1d_rep': np.ascontiguousarray(np.tile(a1d, (128, 1))),
        'a2s_rep': np.ascontiguousarray(np.tile(a2s, (128, 1))),
        'a2d_rep': np.ascontiguousarray(np.tile(a2d, (128, 1))),
        'b1_rep': np.ascontiguousarray(np.tile(b1, (128, 1))),
        'b2_rep': np.ascontiguousarray(np.tile(b2, (128, 1))),
        'iota128': np.ascontiguousarray(
            np.tile(np.arange(128, dtype=np.float32), (128, 1))),
    }
    in_maps = []
    for c in range(cfg.CORES):
        m = dict(shared)
        m['xT'] = np.ascontiguousarray(x[c * NPC:(c + 1) * NPC].T)
        m.update(percore[c])
        in_maps.append(m)
    return in_maps


# ---------------------------------------------------------------------------
# Device program
# ---------------------------------------------------------------------------

def _mid_bcast(ap2d: AP, T: int) -> AP:
    """[128, W] -> [128, T(stride 0), W] view."""
    return AP(ap2d.tensor, ap2d.offset, [ap2d.ap[0], [0, T], ap2d.ap[1]])


def build_program(cfg: Cfg, T_LO, T_HI):
    c = cfg
    DT, NPC, F0, F1, H1, C1, F2, ST1, ST2, KT = (
        c.DT, c.NPC, c.F0, c.F1, c.H1, c.C1, c.F2, c.ST1, c.ST2, c.KT)
    NS_LO, NS_HI = DT * T_LO, DT * T_HI

    nc = bacc.Bacc('TRN2', target_bir_lowering=False, debug=False,
                   num_devices=c.CORES)

    # --- I/O -------------------------------------------------------------
    d_xT = nc.dram_tensor('xT', [F0, NPC], F32, kind='ExternalInput')
    d_W1 = nc.dram_tensor('W1', [F0, F1], F32, kind='ExternalInput')
    d_W2 = nc.dram_tensor('W2', [F1, F2], F32, kind='ExternalInput')
    d_reps = {}
    for nm, w in (('a1s_rep', F1), ('a1d_rep', F1), ('b1_rep', F1),
                  ('a2s_rep', F2), ('a2d_rep', F2), ('b2_rep', F2),
                  ('iota128', 128)):
        d_reps[nm] = nc.dram_tensor(nm, [128, w], F32, kind='ExternalInput')
    d_idx = {}
    for tag, ns in (('lo', NS_LO), ('hi', NS_HI)):
        d_idx['gidx_' + tag] = nc.dram_tensor(
            'gidx_' + tag, [128, ns * 8], I16, kind='ExternalInput')
        d_idx['didx_' + tag] = nc.dram_tensor(
            'didx_' + tag, [128, ns * 8], I16, kind='ExternalInput')
        d_idx['dstrel_' + tag] = nc.dram_tensor(
            'dstrel_' + tag, [128, ns], F32, kind='ExternalInput')
    d_out = nc.dram_tensor('out', [NPC, F2], F32, kind='ExternalOutput')

    # internal DRAM
    t1own = nc.dram_tensor('t1own', [NPC, ST1], F32, kind='Internal')
    t2own = nc.dram_tensor('t2own', [NPC, ST2], F32, kind='Internal')
    al1own = nc.dram_tensor('al1own', [NPC, c.ALS], F32, kind='Internal')
    al2own = nc.dram_tensor('al2own', [NPC, c.ALS], F32, kind='Internal')
    table1 = nc.dram_tensor('table1', [c.N, ST1], F32, kind='Internal',
                            addr_space='Shared')
    table2 = nc.dram_tensor('table2', [c.N, ST2], F32, kind='Internal',
                            addr_space='Shared')

    rg = [list(range(c.CORES))]

    def _body(tc, S):
            nc.gpsimd.load_library(_mlp_lib)
            P = S.enter_context(tc.tile_pool(name='persist', bufs=1))

            # persistent SBUF constants / index arrays
            sb = {}
            W1sb = P.tile([128, KT, F1], F32, tag='W1sb')
            for k in range(KT):
                nc.sync.dma_start(W1sb[:, k, :], d_W1[k * 128:(k + 1) * 128, :])
            W2sb = P.tile([128, F2], F32, tag='W2sb')
            nc.sync.dma_start(W2sb[:], d_W2[:, :])
            for nm in d_reps:
                w = 128 if nm == 'iota128' else (F1 if '1' in nm else F2)
                sb[nm] = P.tile([128, w], F32, tag=nm, name=nm)
                nc.sync.dma_start(sb[nm][:], d_reps[nm][:, :])
            for tag, ns in (('lo', NS_LO), ('hi', NS_HI)):
                for pre, dt_, wmul in (('gidx_', I16, 8), ('didx_', I16, 8),
                                       ('dstrel_', F32, 1)):
                    nm = pre + tag
                    sb[nm] = P.tile([128, ns * wmul], dt_, tag=nm, name=nm)
                    nc.sync.dma_start(sb[nm][:], d_idx[nm][:, :])
            ident = P.tile([128, 128], F32, tag='ident')
            make_identity(nc, ident[:])
            ones = P.tile([128, 1], F32, tag='ones')
            nc.vector.memset(ones[:], 1.0)
            # transposed layer-1 output; own pool so it frees before layer 2
            h1lt_cm = tc.tile_pool(name='h1lt', bufs=1)
            h1lt_pool = h1lt_cm.__enter__()
            h1LT = h1lt_pool.tile([128, DT * 128], F32, tag='h1LT')

            # ---------------- Phase 1: GEMM1 + al1 table -----------------
            with ExitStack() as S1:
                xp = S1.enter_context(tc.tile_pool(name='xslab', bufs=1))
                p1 = S1.enter_context(tc.tile_pool(name='p1sb', bufs=3))
                pp1 = S1.enter_context(
                    tc.tile_pool(name='p1ps', bufs=4, space='PSUM'))
                xTsb = xp.tile([128, KT, NPC], F32)
                for k in range(KT):
                    nc.sync.dma_start(xTsb[:, k, :],
                                      d_xT[k * 128:(k + 1) * 128, :])
                for m in range(DT):
                    c0 = m * 128
                    ph = min(128, NPC - c0)
                    ps = pp1.tile([128, F1], F32, space='PSUM')
                    for k in range(KT):
                        nc.tensor.matmul(ps[:ph, :],
                                         lhsT=xTsb[:, k, c0:c0 + ph],
                                         rhs=W1sb[:, k, :],
                                         start=(k == 0), stop=(k == KT - 1))
                    h1sb = p1.tile([128, F1], F32, tag='h1sb')
                    nc.vector.tensor_copy(h1sb[:ph, :], ps[:ph, :])
                    nc.sync.dma_start(t1own[c0:c0 + ph, 0:F1], h1sb[:ph, :])
                    scr = p1.tile([128, F1], F32, tag='scr')
                    alsv = p1.tile([128, 64], F32, tag='alsv')
                    aldv = p1.tile([128, 64], F32, tag='aldv')
                    nc.vector.memset(alsv[:], 0.0)
                    nc.vector.memset(aldv[:], 0.0)
                    nc.vector.tensor_tensor(scr[:ph, :], h1sb[:ph, :],
                                            sb['a1s_rep'][:ph, :], op=OP.mult)
                    nc.vector.tensor_reduce(
                        alsv[:ph, 0:H1],
                        scr[:ph, :].rearrange('p (h c) -> p h c', h=H1),
                        axis=mybir.AxisListType.X, op=OP.add)
                    nc.vector.tensor_tensor(scr[:ph, :], h1sb[:ph, :],
                                            sb['a1d_rep'][:ph, :], op=OP.mult)
                    nc.vector.tensor_reduce(
                        aldv[:ph, 0:H1],
                        scr[:ph, :].rearrange('p (h c) -> p h c', h=H1),
                        axis=mybir.AxisListType.X, op=OP.add)
                    nc.sync.dma_start(t1own[c0:c0 + ph, F1:ST1],
                                      alsv[:ph, 0:ST1 - F1])
                    nc.sync.dma_start(al1own[c0:c0 + ph, :], aldv[:ph, :])

            _stop = os.environ.get('SPGAT_STOP', 'full')

            def _dbg_out(src_dram, rows, width):
                dp = tc.tile_pool(name='dbg', bufs=1)
                with dp as dpp:
                    for r0 in range(0, rows, 128):
                        pr = min(128, rows - r0)
                        t_ = dpp.tile([128, width], F32, tag='dbgt', name='dbgt')
                        nc.sync.dma_start(t_[:pr, :], src_dram[r0:r0 + pr, 0:width])
                        nc.sync.dma_start(
                            d_out[r0:r0 + pr, 0:min(width, F2)],
                            t_[:pr, 0:min(width, F2)])

            if _stop == 'p1':
                _dbg_out(t1own, NPC, min(ST1, F2))
                h1lt_cm.__exit__(None, None, None)
                return

            # ---------------- Phase 2: AllGather table1 ------------------
            nc.gpsimd.collective_compute(
                'AllGather', OP.bypass, replica_groups=rg,
                ins=[t1own[:, :]], outs=[table1[:, :]])
            if _stop == 'ag1':
                _dbg_out(table1[NPC:2 * NPC, :], NPC, min(ST1, F2))
                h1lt_cm.__exit__(None, None, None)
                return

            # ---------------- Phases 3 & 6: aggregation ------------------
            def aggregate(table, al_own, ST, F, H, layer):
                """Per-edge gather + one-hot-matmul segment softmax."""
                n_half = (c.HALF, c.N - c.HALF)
                streams = (('lo', T_LO, n_half[0]), ('hi', T_HI, n_half[1]))
                CDn = c.CD if layer == 1 else max(1, c.CD // 2)
                with ExitStack() as SA:
                    gp, cp, sp = {}, {}, {}
                    for tag, T_S, _ in streams:
                        gp[tag] = SA.enter_context(tc.tile_pool(
                            name=f'g{layer}{tag}', bufs=2))
                        cp[tag] = SA.enter_context(tc.tile_pool(
                            name=f'c{layer}{tag}', bufs=2))
                        sp[tag] = SA.enter_context(tc.tile_pool(
                            name=f's{layer}{tag}', bufs=2))
                    up = SA.enter_context(tc.tile_pool(
                        name=f'u{layer}', bufs=3, space='PSUM'))
                    up2 = SA.enter_context(tc.tile_pool(
                        name=f'us{layer}', bufs=3, space='PSUM'))
                    fp = SA.enter_context(tc.tile_pool(name=f'f{layer}', bufs=3))
                    ptp = SA.enter_context(tc.tile_pool(
                        name=f'pt{layer}', bufs=2, space='PSUM'))

                    n_chunks = (DT + CDn - 1) // CDn
                    for ch in range(n_chunks):
                        t0 = ch * CDn
                        nd = min(CDn, DT - t0)
                        bufs = {}
                        for tag, T_S, nrows in streams:
                            cd = nd * T_S
                            a = t0 * T_S          # first tile slot
                            ni = cd * 128
                            Hc = gp[tag].tile([128, cd, F], F32, tag='H' + tag)
                            # gather h rows from the half-table
                            half_off = 0 if tag == 'lo' else c.HALF
                            nc.gpsimd.dma_gather(
                                Hc[:, :, :],
                                table[half_off:half_off + nrows, 0:F],
                                sb['gidx_' + tag][:, a * 8:(a + cd) * 8],
                                ni, ni, F, elem_step=ST, single_packet=False)
                            # al_s rows (64-float tail columns of the table)
                            alst = sp[tag].tile([128, cd, 64], F32,
                                                tag='als' + tag)
                            nc.gpsimd.dma_gather(
                                alst[:, :, :],
                                table[half_off:half_off + nrows, F:F + 64],
                                sb['gidx_' + tag][:, a * 8:(a + cd) * 8],
                                ni, ni, 64, elem_step=ST, single_packet=False)
                            # al_d rows from the core-local table, by dst_local
                            aldt = sp[tag].tile([128, cd, 64], F32,
                                                tag='ald' + tag)
                            nc.gpsimd.dma_gather(
                                aldt[:, :, :], al_own[:, :],
                                sb['didx_' + tag][:, a * 8:(a + cd) * 8],
                                ni, ni, 64, elem_step=64, single_packet=False)
                            lsum = sp[tag].tile([128, cd, H], F32, tag='ls' + tag)
                            nc.vector.tensor_tensor(lsum[:, :, :],
                                                    alst[:, :, 0:H],
                                                    aldt[:, :, 0:H], op=OP.add)
                            lk = sp[tag].tile([128, cd, H], F32, tag='lk' + tag)
                            nc.vector.scalar_tensor_tensor(
                                lk[:, :, :], lsum[:, :, :], NEG_SLOPE,
                                lsum[:, :, :], op0=OP.mult, op1=OP.max)
                            ee = sp[tag].tile([128, cd, H], F32, tag='ee' + tag)
                            nc.scalar.activation(ee[:, :, :], lk[:, :, :], AF.Exp)
                            cmp = cp[tag].tile([128, cd, 128], F32, tag='cmp' + tag)
                            drel_v = sb['dstrel_' + tag][:, a:a + cd] \
                                .to_broadcast([128, cd, 128])
                            iota_v = _mid_bcast(sb['iota128'][:, :], cd)
                            nc.vector.tensor_tensor(cmp[:, :, :], drel_v, iota_v,
                                                    op=OP.is_equal)
                            if layer == 1:
                                # scale gathered rows by ee (per head) in place
                                Hv = Hc[:, :, :].rearrange(
                                    'p t (h cc) -> p t h cc', h=H)
                                nc.vector.tensor_tensor(
                                    Hv, Hv, ee[:, :, :].to_broadcast(
                                        [128, cd, H, F // H]), op=OP.mult)
                            else:
                                # fold ee into the one-hot lhsT instead
                                nc.vector.tensor_tensor(
                                    cmp[:, :, :], cmp[:, :, :],
                                    ee[:, :, :].rearrange('p t h -> p (t h)')
                                    .to_broadcast([128, cd, 128]), op=OP.mult)
                            bufs[tag] = (Hc, cmp, ee, T_S)

                        for tt_ in range(t0, t0 + nd):
                            U = up.tile([128, F], F32, space='PSUM')
                            sU = up2.tile([128, H], F32, space='PSUM')
                            n_mm = sum(T_S for _, T_S, _ in streams)
                            mm_i = 0
                            for tag, T_S, _ in streams:
                                Hc, cmp, ee, _ = bufs[tag]
                                for j in range(T_S):
                                    jj = (tt_ - t0) * T_S + j
                                    first = mm_i == 0
                                    last = mm_i == n_mm - 1
                                    nc.tensor.matmul(
                                        U[:, :], lhsT=cmp[:, jj, :],
                                        rhs=Hc[:, jj, :],
                                        start=first, stop=last)
                                    nc.tensor.matmul(
                                        sU[:, :], lhsT=cmp[:, jj, :],
                                        rhs=(ee[:, jj, :] if layer == 1
                                             else ones[:, :]),
                                        start=first, stop=last)
                                    mm_i += 1
                            c0 = tt_ * 128
                            ph = min(128, NPC - c0)
                            s_t = fp.tile([128, H], F32, tag='s')
                            nc.vector.tensor_scalar(
                                s_t[:, :], sU[:, :], 1e-30, None,
                                op0=OP.max)
                            rec = fp.tile([128, H], F32, tag='rec')
                            nc.vector.reciprocal(rec[:, :], s_t[:, :])
                            hL = fp.tile([128, F], F32, tag='hL')
                            nc.vector.tensor_tensor(
                                hL[:, :].rearrange('p (h cc) -> p h cc', h=H),
                                U[:, :].rearrange('p (h cc) -> p h cc', h=H),
                                rec[:, :].to_broadcast([128, H, F // H]),
                                op=OP.mult)
                            if layer == 1:
                                nc.vector.tensor_tensor(hL[:, :], hL[:, :],
                                                        sb['b1_rep'][:, :],
                                                        op=OP.add)
                                pt = ptp.tile([128, 128], F32, space='PSUM')
                                nc.tensor.transpose(pt[:, :], hL[:, :],
                                                    ident[:, :])
                                nc.vector.tensor_copy(
                                    h1LT[:, tt_ * 128:(tt_ + 1) * 128], pt[:, :])
                            else:
                                nc.vector.tensor_tensor(hL[:, :], hL[:, :],
                                                        sb['b2_rep'][:, :],
                                                        op=OP.add)
                                if os.environ.get('SPGAT_L2MODE') == 'nonorm':
                                    nc.sync.dma_start(d_out[c0:c0 + ph, :],
                                                      hL[:ph, :])
                                    continue
                                scr2 = fp.tile([128, F], F32, tag='scr2')
                                ss = fp.tile([128, 1], F32, tag='ss')
                                nc.vector.tensor_tensor(scr2[:, :], hL[:, :],
                                                        hL[:, :], op=OP.mult)
                                nc.vector.tensor_reduce(
                                    ss[:, :], scr2[:, :],
                                    axis=mybir.AxisListType.X, op=OP.add)
                                nrm = fp.tile([128, 1], F32, tag='nrm')
                                nc.scalar.sqrt(nrm[:, :], ss[:, :])
                                nc.vector.tensor_scalar(
                                    nrm[:, :], nrm[:, :], 1e-12, None,
                                    op0=OP.max)
                                rc2 = fp.tile([128, 1], F32, tag='rc2')
                                nc.vector.reciprocal(rc2[:, :], nrm[:, :])
                                ot = fp.tile([128, F], F32, tag='ot')
                                nc.vector.tensor_scalar_mul(
                                    ot[:, :], hL[:, :], rc2[:, :1])
                                nc.vector.tensor_scalar_max(
                                    ot[:, :], ot[:, :], 0.0)
                                nc.sync.dma_start(d_out[c0:c0 + ph, :],
                                                  ot[:ph, :])

            aggregate(table1, al1own, ST1, F1, H1, layer=1)
            if _stop == 'l1':
                # dump h1LT[:, 0:F2] (feat x first-F2-nodes) into out[0:128]
                nc.sync.dma_start(d_out[0:128, 0:F2], h1LT[:, 0:F2])
                h1lt_cm.__exit__(None, None, None)
                return

            # ---------------- Phase 4: GEMM2 + al2 table -----------------
            with ExitStack() as S4:
                p4 = S4.enter_context(tc.tile_pool(name='p4sb', bufs=3))
                pp4 = S4.enter_context(
                    tc.tile_pool(name='p4ps', bufs=4, space='PSUM'))
                for m in range(DT):
                    c0 = m * 128
                    ph = min(128, NPC - c0)
                    ps = pp4.tile([128, F2], F32, space='PSUM')
                    nc.tensor.matmul(ps[:ph, :], lhsT=h1LT[:, c0:c0 + ph],
                                     rhs=W2sb[:, :], start=True, stop=True)
                    h2sb = p4.tile([128, F2], F32, tag='h2sb')
                    nc.vector.tensor_copy(h2sb[:ph, :], ps[:ph, :])
                    nc.sync.dma_start(t2own[c0:c0 + ph, 0:F2], h2sb[:ph, :])
                    scr = p4.tile([128, F2], F32, tag='scr4')
                    alsv = p4.tile([128, 64], F32, tag='alsv4')
                    aldv = p4.tile([128, 64], F32, tag='aldv4')
                    nc.vector.memset(alsv[:], 0.0)
                    nc.vector.memset(aldv[:], 0.0)
                    nc.vector.tensor_tensor(scr[:ph, :], h2sb[:ph, :],
                                            sb['a2s_rep'][:ph, :], op=OP.mult)
                    nc.vector.tensor_reduce(alsv[:ph, 0:1], scr[:ph, :],
                                            axis=mybir.AxisListType.X, op=OP.add)
                    nc.vector.tensor_tensor(scr[:ph, :], h2sb[:ph, :],
                                            sb['a2d_rep'][:ph, :], op=OP.mult)
                    nc.vector.tensor_reduce(aldv[:ph, 0:1], scr[:ph, :],
                                            axis=mybir.AxisListType.X, op=OP.add)
                    nc.sync.dma_start(t2own[c0:c0 + ph, F2:ST2],
                                      alsv[:ph, 0:ST2 - F2])
                    nc.sync.dma_start(al2own[c0:c0 + ph, :], aldv[:ph, :])

            if _stop == 'p4':
                _dbg_out(t2own, NPC, F2)
                h1lt_cm.__exit__(None, None, None)
                return
            # ---------------- Phase 5: AllGather table2 ------------------
            h1lt_cm.__exit__(None, None, None)
            nc.gpsimd.collective_compute(
                'AllGather', OP.bypass, replica_groups=rg,
                ins=[t2own[:, :]], outs=[table2[:, :]])

            if _stop == 'ag2':
                _dbg_out(table2[NPC:2 * NPC, :], NPC, F2)
                return
            if _stop == 'ag2b':
                _dbg_out(t2own, NPC, F2)
                return
            aggregate(table2, al2own, ST2, F2, 1, layer=2)

    with tile.TileContext(nc) as tc:
        with ExitStack() as S:
            _body(tc, S)
    nc.compile()
    return nc


# ---------------------------------------------------------------------------
# Entry point
# ---------------------------------------------------------------------------

_BUILD_CACHE = {}


def _get_program(cfg, T_LO, T_HI):
    key = (cfg.N, cfg.E, cfg.CORES, T_LO, T_HI, cfg.CD)
    if key not in _BUILD_CACHE:
        _BUILD_CACHE[key] = build_program(cfg, T_LO, T_HI)
    return _BUILD_CACHE[key]


def kernel(**inputs) -> np.ndarray:
    x = np.asarray(inputs['x'])
    edge_index = np.asarray(inputs['edge_index'])
    n = x.shape[1]
    cfg = Cfg(N=n, E=edge_index.shape[1])
    percore, T_LO, T_HI = preprocess(edge_index, cfg)
    nc = _get_program(cfg, T_LO, T_HI)
    in_maps = make_in_maps(inputs, cfg, percore, T_LO, T_HI)
    res = run_bass_kernel_spmd(nc, in_maps, core_ids=list(range(cfg.CORES)))
    out = np.concatenate([r['out'] for r in res.results], axis=0)
    return out.reshape(1, n, cfg.F2).astype(np.float32)
